# revision 6
# baseline (speedup 1.0000x reference)
"""Trainium2 Bass kernel for nn_CaptionModel (GRU caption decoder).

Model: h0 = feat; x0 = embed[<SOS>]; 200 GRU steps where the output hidden
state is fed back as the next input (x_t = h_t for t >= 1), then a linear
projection of every hidden state to vocab logits, output [B, V, T].

Since x_t == h_t for t >= 1 the two GRU matmuls fuse into one 2048-wide
gate matmul G = h @ Wc.T + bc with Wc = [w_ih_r+w_hh_r; w_ih_z+w_hh_z;
w_hh_n; w_ih_n], gates r = sig(G0), z = sig(G1), n = tanh(G3 + r*G2),
h' = (1-z)*n + z*h.

Layout: GATE-MAJOR, fp16 matmuls, stationary 128x128 weight blocks with
the small hidden state as moving operand.  h' = u + e is split into u =
z*h and e = (1-z)*n fed as TWO accumulating moving operands so the PE
can start on u while the tanh chain still runs.

Key scheduling idea vs the single-bank version: the per-step gate PSUM
is split across THREE banks (rz / hn / in, 2-step parity -> 6 banks,
plus 2 proj banks = 8).  The matmul burst is ordered [u_rz, e_rz, u_hn,
e_hn, u_in(q01), e_in(q01), u_in(q23), e_in(q23), bias(t+1), proj] and
the r-sigmoid fires as soon as the rz bank stops (832ns into the burst)
instead of after the whole 1.9us burst.  The z gate is handled by two
extra Act ops: z = sig(-zpre_psum) (scale=-1; z weights are negated
host-side so the psum holds -zpre) feeds a single u = z*h mul on Pool,
and z1m = sig(zpre_psum) = 1-z feeds the e-mul.  tanh + e-mul run in
q-halves so e(c01) unblocks the next burst's e_rz(c01) matmuls early.

Sharding: pure data parallelism, batch 256 -> 32 per core on 8 cores,
weights replicated.
"""

import os
from contextlib import ExitStack

import numpy as np

import concourse.bass as bass
import concourse.tile as tile
from concourse import bacc, mybir
from concourse.bass_utils import run_bass_kernel_spmd

B, H, VOCAB = 256, 512, 100
STEPS = int(os.environ.get("KERNEL_STEPS", "200"))
NCORES = 8
BD = B // NCORES  # 32
F16 = mybir.dt.float16
F32 = mybir.dt.float32
SIG = mybir.ActivationFunctionType.Sigmoid
TANH = mybir.ActivationFunctionType.Tanh

# gate order in the stationary weight blocks: r z hn in
GATES = ("r", "z", "hn", "in")
GI = {g: i for i, g in enumerate(GATES)}
PROJ_EVERY = 4


def _blk(g, q, c):
    return ((GI[g] * 4 + q) * 4 + c) * 128


def _build(steps: int):
    nc = bacc.Bacc("TRN2", target_bir_lowering=False, debug=False,
                   num_devices=NCORES)
    T1 = steps + 1

    wst_d = nc.dram_tensor("wst", [128, 64 * 128], F16, kind="ExternalInput").ap()
    wst0_d = nc.dram_tensor("wst0", [128, 32 * 128], F16, kind="ExternalInput").ap()
    h0_d = nc.dram_tensor("h0", [128, 128], F16, kind="ExternalInput").ap()
    bt_d = nc.dram_tensor("bt", [1, 2048], F16, kind="ExternalInput").ap()
    bt0_d = nc.dram_tensor("bt0", [1, 2048], F16, kind="ExternalInput").ap()
    ones_d = nc.dram_tensor("ones", [1, BD], F16, kind="ExternalInput").ap()
    pjt_d = nc.dram_tensor("pjt", [128, 4 * VOCAB], F16, kind="ExternalInput").ap()
    pjb_d = nc.dram_tensor("pjb", [VOCAB, 1], F32, kind="ExternalInput").ap()
    out_d = nc.dram_tensor("out", [BD, VOCAB, steps], F32,
                           kind="ExternalOutput").ap()

    with tile.TileContext(nc) as tc, ExitStack() as ctx:
        sg = ctx.enter_context(tc.tile_pool(name="sg", bufs=1))
        wk = ctx.enter_context(tc.tile_pool(name="wk", bufs=3))

        wst = sg.tile([128, 64 * 128], F16)
        nc.sync.dma_start(out=wst, in_=wst_d)
        wst0 = sg.tile([128, 32 * 128], F16)
        nc.sync.dma_start(out=wst0, in_=wst0_d)
        bt = sg.tile([1, 2048], F16)
        nc.sync.dma_start(out=bt, in_=bt_d)
        bt0 = sg.tile([1, 2048], F16)
        nc.sync.dma_start(out=bt0, in_=bt0_d)
        ones = sg.tile([1, BD], F16)
        nc.sync.dma_start(out=ones, in_=ones_d)
        pjt = sg.tile([128, 4 * VOCAB], F16)
        nc.sync.dma_start(out=pjt, in_=pjt_d)
        pjb = sg.tile([VOCAB, 1], F32)
        nc.sync.dma_start(out=pjb, in_=pjb_d)
        hist = sg.tile([128, 4, T1, BD], F16, name="hist")
        nc.sync.dma_start(out=hist[:, :, 0, :],
                          in_=h0_d.rearrange("p (q b) -> p q b", q=4))
        stage = sg.tile([VOCAB, BD * steps], F32, name="stage")

        with tc.tile_pool(name="gps", bufs=1, space="PSUM") as gpool, \
             tc.tile_pool(name="pps", bufs=2, space="PSUM") as ppool:
            # Each region gets a FULL 2KB psum bank (tile padded to 512 f32
            # cols) so start=True's whole-bank zeroing can't touch a
            # neighbouring region.  2-step parity rotation.
            RZ = [gpool.tile([128, 512], F32, tag=f"RZ{i}", name=f"RZ{i}")
                  for i in range(2)]
            HN = [gpool.tile([128, 512], F32, tag=f"HN{i}", name=f"HN{i}")
                  for i in range(2)]
            IN = [gpool.tile([128, 512], F32, tag=f"IN{i}", name=f"IN{i}")
                  for i in range(2)]
            started = {}  # (bank_name) -> whether bank's group was started

            def bank_of(t, gate):
                p = t % 2
                if gate in ("r", "z"):
                    return RZ[p]
                return HN[p] if gate == "hn" else IN[p]

            def col_of(gate, q):
                # r at cols 0:128, z at 128:256 of the rz bank; hn/in at
                # 0:128 of their own banks
                base = 128 if gate == "z" else 0
                return base + q * BD

            def bias_mms(t, bias, gates, stop_banks=()):
                # bias matmuls for step t's banks; the first mm into each
                # bank carries start=True (zeroes the bank); banks named in
                # stop_banks end their accumulation group on their last
                # bias mm (used for the t=0 bias-only in bank)
                emitted = [(gate, q) for gate in gates for q in range(4)]
                last_per_bank = {}
                for i, (gate, q) in enumerate(emitted):
                    bname = "rz" if gate in ("r", "z") else gate
                    if bname in stop_banks:
                        last_per_bank[bname] = i
                for i, (gate, q) in enumerate(emitted):
                    g = bank_of(t, gate)
                    bname = "rz" if gate in ("r", "z") else gate
                    key = (t, bname)
                    first = not started.get(key, False)
                    started[key] = True
                    col = col_of(gate, q)
                    nc.tensor.matmul(
                        g[:, col:col + BD],
                        bias[:, GI[gate] * 512 + q * 128:
                             GI[gate] * 512 + (q + 1) * 128],
                        ones, start=first,
                        stop=(last_per_bank.get(bname) == i),
                        skip_group_check=True)

            def w_mms(t, w, rhs4, gates, cs=(0, 1, 2, 3), qs=(0, 1, 2, 3),
                      stop_at_end=()):
                # weight matmuls into step t's banks; stop_at_end names the
                # banks whose accumulation group ends with this batch
                emitted = []
                for gate in gates:
                    for q in qs:
                        for c in cs:
                            emitted.append((gate, q, c))
                last_per_bank = {}
                for i, (gate, q, c) in enumerate(emitted):
                    bname = "rz" if gate in ("r", "z") else gate
                    if bname in stop_at_end:
                        last_per_bank[bname] = i
                for i, (gate, q, c) in enumerate(emitted):
                    g = bank_of(t, gate)
                    col = col_of(gate, q)
                    if w is wst0 and gate in ("r", "z"):
                        wt, base = wst0, ((GI[gate] * 4 + q) * 4 + c) * 128
                    else:
                        wt, base = wst, _blk(gate, q, c)
                    bname = "rz" if gate in ("r", "z") else gate
                    stop = last_per_bank.get(bname) == i
                    nc.tensor.matmul(
                        g[:, col:col + BD],
                        wt[:, base:base + 128],
                        rhs4[c], start=False, stop=stop,
                        skip_group_check=True)

            def proj_rows(r0, nrows):
                # P free dim iterates (b, t) so the stage (b-major,
                # t-minor) write and the final DMA stay contiguous in t
                Pfull = ppool.tile([VOCAB, 512], F32, tag="P", name="Pfull")
                P = Pfull[:, 0:BD * nrows]
                for c in range(4):
                    rhs = hist[:, c, r0:r0 + nrows, :].rearrange(
                        "p t b -> p b t")
                    nc.tensor.matmul(P, pjt[:, c * VOCAB:(c + 1) * VOCAB], rhs,
                                     start=(c == 0), stop=(c == 3))
                st_sl = stage.rearrange("p (b t) -> p b t", b=BD)[
                    :, :, r0 - 1:r0 - 1 + nrows]
                nc.vector.tensor_scalar_add(
                    st_sl, P.rearrange("p (b t) -> p b t", b=BD), pjb)

            # ---- step 0: bias0 + w_hh matmuls on h0 (in-region bias-only)
            h0c = [hist[:, c, 0, :] for c in range(4)]
            bias_mms(0, bt0, ("r", "z", "hn", "in"), stop_banks=("in",))
            w_mms(0, wst0, h0c, ("r", "z"), stop_at_end=("rz",))
            w_mms(0, wst, h0c, ("hn",), stop_at_end=("hn",))
            if steps > 1:
                bias_mms(1, bt, ("r", "z", "hn", "in"))

            next_proj = 1
            dma_done = 0
            DMA_CHUNK = 50
            for t in range(steps):
                p = t % 2
                rz_s = wk.tile([128, 256], F32, tag="rz")
                z_s = wk.tile([128, 128], F32, tag="z")
                a_s = wk.tile([128, 128], F32, tag="a")
                b_s = wk.tile([128, 128], F32, tag="b")
                n16 = wk.tile([128, 128], F16, tag="n")
                u16 = wk.tile([128, 4, BD], F16, tag="u")
                e16 = wk.tile([128, 4, BD], F16, tag="e")

                # ---- matmul burst for step t+1 gates (emitted while step
                # t's chain runs) happens at the tail of this loop body; the
                # chain for step t reads the banks filled by the previous
                # iteration's burst (or the step-0 prologue above).
                RZb, HNb, INb = RZ[p], HN[p], IN[p]

                # Act: r halves first (feed the a->b->tanh->e chain), then
                # direct z (feeds u = z*h on Pool), then z1m (feeds e-mul).
                nc.scalar.activation(rz_s[:, 0:64], RZb[:, 0:64], SIG)
                nc.scalar.activation(rz_s[:, 64:128], RZb[:, 64:128], SIG)
                nc.scalar.activation(z_s, RZb[:, 128:256], SIG, scale=-1.0)
                nc.scalar.activation(rz_s[:, 128:256], RZb[:, 128:256], SIG)

                z4 = z_s.rearrange("p (q b) -> p q b", q=4)
                z1m4 = rz_s[:, 128:256].rearrange("p (q b) -> p q b", q=4)
                n4 = n16.rearrange("p (q b) -> p q b", q=4)

                # u = z * h on Pool (single mul; off the critical chain)
                nc.gpsimd.tensor_mul(u16, z4, hist[:, :, t, :])

                # e-chain on DVE/Act in q-halves: a = r*g_hn, b = a+g_in,
                # tanh, e = z1m*n
                nc.vector.tensor_mul(a_s[:, 0:64], rz_s[:, 0:64],
                                     HNb[:, 0:64])
                nc.vector.tensor_add(b_s[:, 0:64], a_s[:, 0:64],
                                     INb[:, 0:64])
                nc.vector.tensor_mul(a_s[:, 64:128], rz_s[:, 64:128],
                                     HNb[:, 64:128])
                nc.vector.tensor_add(b_s[:, 64:128], a_s[:, 64:128],
                                     INb[:, 64:128])
                nc.scalar.activation(n16[:, 0:64], b_s[:, 0:64], TANH)
                nc.scalar.activation(n16[:, 64:128], b_s[:, 64:128], TANH)
                nc.vector.tensor_mul(e16[:, 0:2, :], z1m4[:, 0:2, :],
                                     n4[:, 0:2, :])
                nc.vector.tensor_mul(e16[:, 2:4, :], z1m4[:, 2:4, :],
                                     n4[:, 2:4, :])

                # h join for hist/proj/u-path of the next step (Pool)
                nc.gpsimd.tensor_add(hist[:, :, t + 1, :], u16, e16)

                if t + 1 < steps:
                    u4 = [u16[:, c, :] for c in range(4)]
                    e4 = [e16[:, c, :] for c in range(4)]
                    # burst computing G(t+1): the whole u-side first (u is
                    # ready ~1us before e), then the e-side ordered rz ->
                    # hn -> in so the r-sigmoid fires right after e_rz and
                    # the hn/in banks complete early for the a/b ops
                    w_mms(t + 1, wst, u4, ("r", "z"))
                    w_mms(t + 1, wst, u4, ("hn",))
                    w_mms(t + 1, wst, u4, ("in",))
                    w_mms(t + 1, wst, e4, ("r", "z"), cs=(0, 1))
                    w_mms(t + 1, wst, e4, ("r", "z"), cs=(2, 3),
                          stop_at_end=("rz",))
                    w_mms(t + 1, wst, e4, ("hn",), stop_at_end=("hn",))
                    w_mms(t + 1, wst, e4, ("in",), stop_at_end=("in",))
                    if t + 2 < steps:
                        bias_mms(t + 2, bt, ("r", "z", "hn", "in"))
                    if next_proj + PROJ_EVERY <= t:
                        proj_rows(next_proj, PROJ_EVERY)
                        next_proj += PROJ_EVERY
                        # stream finalized 50-row output chunks during the
                        # recurrence instead of one big DMA at the end
                        while dma_done + DMA_CHUNK < next_proj:
                            lo = dma_done
                            nc.sync.dma_start(
                                out=out_d[:, :, lo:lo + DMA_CHUNK].rearrange(
                                    "b v t -> v b t"),
                                in_=stage.rearrange("p (b t) -> p b t", b=BD)[
                                    :, :, lo:lo + DMA_CHUNK])
                            dma_done += DMA_CHUNK

            while next_proj <= steps:
                nrows = min(PROJ_EVERY, steps + 1 - next_proj)
                proj_rows(next_proj, nrows)
                next_proj += nrows

        if dma_done < steps:
            nc.sync.dma_start(
                out=out_d[:, :, dma_done:].rearrange("b v t -> v b t"),
                in_=stage.rearrange("p (b t) -> p b t", b=BD)[:, :, dma_done:])
    nc.compile()
    return nc


_CACHE = {}


def _get_nc(steps: int):
    if steps not in _CACHE:
        _CACHE[steps] = _build(steps)
    return _CACHE[steps]


def _prep_inputs(feat, embed_table, w_ih, w_hh, b_ih, b_hh, proj_w, proj_b):
    f32 = np.float32
    f16 = np.float16
    w_ih = np.asarray(w_ih, f32)
    w_hh = np.asarray(w_hh, f32)
    b_ih = np.asarray(b_ih, f32)
    b_hh = np.asarray(b_hh, f32)
    # fused gate weights, gate-major order r, z, hn, in
    # z gate negated: sigmoid(z psum) then directly equals 1 - z
    Wc = np.concatenate([w_ih[:H] + w_hh[:H],
                         -(w_ih[H:2 * H] + w_hh[H:2 * H]),
                         w_hh[2 * H:],
                         w_ih[2 * H:]], 0)          # [4H, H]
    bc = np.concatenate([b_ih[:H] + b_hh[:H],
                         -(b_ih[H:2 * H] + b_hh[H:2 * H]),
                         b_hh[2 * H:],
                         b_ih[2 * H:]], 0)          # [4H]

    x0 = np.asarray(embed_table, f32)[0]
    gi0 = w_ih @ x0 + b_ih                          # [3H]
    bc0 = np.concatenate([gi0[:H] + b_hh[:H],
                          -(gi0[H:2 * H] + b_hh[H:2 * H]),
                          b_hh[2 * H:],
                          gi0[2 * H:]], 0)          # [4H]
    W0 = np.concatenate([w_hh[:H], -w_hh[H:2 * H]], 0)  # [2H, H] r,z step-0

    # stationary blocks: wst[kp, ((g*4+q)*4+c)*128 + m] = Wc[g*512+q*128+m,
    #                                                        c*128+kp]
    wst = np.empty((128, 64 * 128), f32)
    for g in range(4):
        for q in range(4):
            for c in range(4):
                blk = ((g * 4 + q) * 4 + c) * 128
                wst[:, blk:blk + 128] = Wc[g * 512 + q * 128:
                                           g * 512 + (q + 1) * 128,
                                           c * 128:(c + 1) * 128].T
    wst0 = np.empty((128, 32 * 128), f32)
    for g in range(2):
        for q in range(4):
            for c in range(4):
                blk = ((g * 4 + q) * 4 + c) * 128
                wst0[:, blk:blk + 128] = W0[g * 512 + q * 128:
                                            g * 512 + (q + 1) * 128,
                                            c * 128:(c + 1) * 128].T

    proj_w = np.asarray(proj_w, f32)                # [V, H]
    pjt = np.empty((128, 4 * VOCAB), f32)
    for c in range(4):
        pjt[:, c * VOCAB:(c + 1) * VOCAB] = proj_w[:, c * 128:(c + 1) * 128].T

    feat = np.asarray(feat, f32)
    common = {
        "wst": wst.astype(f16),
        "wst0": wst0.astype(f16),
        "bt": bc.reshape(1, 2048).astype(f16),
        "bt0": bc0.reshape(1, 2048).astype(f16),
        "ones": np.ones((1, BD), f16),
        "pjt": pjt.astype(f16),
        "pjb": np.asarray(proj_b, f32).reshape(VOCAB, 1),
    }
    maps = []
    for i in range(NCORES):
        fs = feat[i * BD:(i + 1) * BD]              # [BD, H]
        h0g = np.ascontiguousarray(
            fs.T.reshape(4, 128, BD).transpose(1, 0, 2).reshape(128, 128))
        maps.append(dict(common, h0=h0g.astype(f16)))
    return maps


def kernel(feat, embed_table, w_ih, w_hh, b_ih, b_hh, proj_w, proj_b,
           _trace=False):
    nc = _get_nc(STEPS)
    in_maps = _prep_inputs(feat, embed_table, w_ih, w_hh, b_ih, b_hh,
                           proj_w, proj_b)
    res = run_bass_kernel_spmd(nc, in_maps, list(range(NCORES)), trace=_trace)
    out = np.concatenate([res.results[i]["out"] for i in range(NCORES)], 0)
    if _trace:
        kernel.last_exec_time_ns = res.exec_time_ns
        kernel.last_results = res
    return out


# revision 7
# speedup vs baseline: 1.2423x; 1.2423x over previous
"""Trainium2 Bass kernel for nn_CaptionModel (GRU caption decoder).

Model: h0 = feat; x0 = embed[<SOS>]; 200 GRU steps where the output hidden
state is fed back as the next input (x_t = h_t for t >= 1), then a linear
projection of every hidden state to vocab logits, output [B, V, T].

Since x_t == h_t for t >= 1 the two GRU matmuls fuse into one 2048-wide
gate matmul G = h @ Wc.T + bc with Wc = [w_ih_r+w_hh_r; -(w_ih_z+w_hh_z);
w_hh_n; w_ih_n] (z negated so sig(psum) = 1-z), gates r = sig(G0),
z1m = sig(G1) = 1-z, n = tanh(G3 + r*G2), h' = (h - z1m*h) + z1m*n.

KEY STRUCTURE (vs the single-recurrence version): the GRU recurrence is
latency-bound -- the serial loop [gate mms -> sigmoid -> a -> b -> tanh ->
e-mul -> join] crosses engines ~6 times at ~300ns per handoff, so one
recurrence cycles at ~3us regardless of batch.  Since the per-core batch
of 32 is data-parallel, we run TWO independent 16-sample recurrences
phase-shifted by half a cycle, interleaved on the same engines (each
engine is <60% busy).  Throughput doubles: ~1.5us/step effective.

Per half: gate PSUM is 2 single banks (RZ: r|z1m, NI: hn|in) -- PSUM
dependency tracking is bank-granular, so each bank is fully written
(bias first, start=True zeroes it) then read.  The PE consumes the
JOINED hidden h(t+1) = u32 + e32 (64 mms of 16 rows instead of 128) to
keep PE load at ~1us/step for both halves; u32 = h - z1m*h (Pool, f32)
and e32 = z1m*n (DVE, f32) are f32 so h' rounds to fp16 once per step
(fewer roundings than the u16/e16 split).  tanh chain on Act/DVE.

Sharding: pure data parallelism, batch 256 -> 32 per core on 8 cores,
weights replicated.
"""

import os
from contextlib import ExitStack

import numpy as np

import concourse.bass as bass
import concourse.tile as tile
from concourse import bacc, mybir
from concourse.bass_utils import run_bass_kernel_spmd

B, H, VOCAB = 256, 512, 100
STEPS = int(os.environ.get("KERNEL_STEPS", "200"))
NCORES = 8
BD = B // NCORES  # 32
HB = BD // 2      # 16 per interleaved half-recurrence
F16 = mybir.dt.float16
F32 = mybir.dt.float32
SIG = mybir.ActivationFunctionType.Sigmoid
TANH = mybir.ActivationFunctionType.Tanh

# gate order in the stationary weight blocks: r z hn in
GATES = ("r", "z", "hn", "in")
GI = {g: i for i, g in enumerate(GATES)}
PROJ_EVERY = 4


def _blk(g, q, c):
    return ((GI[g] * 4 + q) * 4 + c) * 128


def _colof(g, q):
    # within-bank columns: r/hn at q*HB (cols 0:64), z/in at 64+q*HB
    return (64 if g in ("z", "in") else 0) + q * HB


def _build(steps: int):
    nc = bacc.Bacc("TRN2", target_bir_lowering=False, debug=False,
                   num_devices=NCORES)
    T1 = steps + 1

    wst_d = nc.dram_tensor("wst", [128, 64 * 128], F16, kind="ExternalInput").ap()
    wst0_d = nc.dram_tensor("wst0", [128, 32 * 128], F16, kind="ExternalInput").ap()
    h0_d = nc.dram_tensor("h0", [128, 128], F16, kind="ExternalInput").ap()
    bt_d = nc.dram_tensor("bt", [1, 2048], F16, kind="ExternalInput").ap()
    bt0_d = nc.dram_tensor("bt0", [1, 2048], F16, kind="ExternalInput").ap()
    ones_d = nc.dram_tensor("ones", [1, BD], F16, kind="ExternalInput").ap()
    pjt_d = nc.dram_tensor("pjt", [128, 4 * VOCAB], F16, kind="ExternalInput").ap()
    pjb_d = nc.dram_tensor("pjb", [VOCAB, 1], F32, kind="ExternalInput").ap()
    out_d = nc.dram_tensor("out", [BD, VOCAB, steps], F32,
                           kind="ExternalOutput").ap()

    with tile.TileContext(nc) as tc, ExitStack() as ctx:
        sg = ctx.enter_context(tc.tile_pool(name="sg", bufs=1))
        wk = ctx.enter_context(tc.tile_pool(name="wk", bufs=3))

        wst = sg.tile([128, 64 * 128], F16)
        nc.sync.dma_start(out=wst, in_=wst_d)
        wst0 = sg.tile([128, 32 * 128], F16)
        nc.sync.dma_start(out=wst0, in_=wst0_d)
        bt = sg.tile([1, 2048], F16)
        nc.sync.dma_start(out=bt, in_=bt_d)
        bt0 = sg.tile([1, 2048], F16)
        nc.sync.dma_start(out=bt0, in_=bt0_d)
        ones = sg.tile([1, BD], F16)
        nc.sync.dma_start(out=ones, in_=ones_d)
        pjt = sg.tile([128, 4 * VOCAB], F16)
        nc.sync.dma_start(out=pjt, in_=pjt_d)
        pjb = sg.tile([VOCAB, 1], F32)
        nc.sync.dma_start(out=pjb, in_=pjb_d)
        hist = sg.tile([128, 4, T1, BD], F16, name="hist")
        nc.sync.dma_start(out=hist[:, :, 0, :],
                          in_=h0_d.rearrange("p (q b) -> p q b", q=4))
        stage = sg.tile([VOCAB, BD * steps], F32, name="stage")

        with tc.tile_pool(name="gps", bufs=1, space="PSUM") as gpool, \
             tc.tile_pool(name="pps", bufs=2, space="PSUM") as ppool:
            # one RZ + one NI bank per half (full 2KB banks; bank-granular
            # deps + whole-bank zeroing mean banks can't be shared)
            RZ = [gpool.tile([128, 512], F32, tag=f"RZ{i}", name=f"RZ{i}")
                  for i in range(2)]
            NI = [gpool.tile([128, 512], F32, tag=f"NI{i}", name=f"NI{i}")
                  for i in range(2)]

            def bank_mms(X, bank, gates, bias_src, w_src, row, wgates=None):
                # fill one bank for half X: bias mms (first carries start)
                # then weight mms on hist[:, c, row, half] (last carries
                # stop).  wgates limits which gates get weight mms (t=0
                # in-region is bias-only).
                hb0 = X * HB
                if wgates is None:
                    wgates = gates
                first = True
                for g in gates:
                    for q in range(4):
                        col = _colof(g, q)
                        nc.tensor.matmul(
                            bank[:, col:col + HB],
                            bias_src[:, GI[g] * 512 + q * 128:
                                     GI[g] * 512 + (q + 1) * 128],
                            ones[:, 0:HB], start=first, stop=False,
                            skip_group_check=True)
                        first = False
                mms = [(g, q, c) for g in wgates for q in range(4)
                       for c in range(4)]
                for i, (g, q, c) in enumerate(mms):
                    if w_src is wst0 and g in ("r", "z"):
                        wt = wst0
                    else:
                        wt = wst
                    base = _blk(g, q, c)
                    col = _colof(g, q)
                    nc.tensor.matmul(
                        bank[:, col:col + HB],
                        wt[:, base:base + 128],
                        hist[:, c, row, hb0:hb0 + HB],
                        start=False, stop=(i == len(mms) - 1),
                        skip_group_check=True)

            def proj_rows(X, r0, nrows):
                hb0 = X * HB
                Pfull = ppool.tile([VOCAB, 512], F32, tag="P", name="Pfull")
                P = Pfull[:, 0:HB * nrows]
                for c in range(4):
                    rhs = hist[:, c, r0:r0 + nrows, hb0:hb0 + HB].rearrange(
                        "p t b -> p b t")
                    nc.tensor.matmul(P, pjt[:, c * VOCAB:(c + 1) * VOCAB], rhs,
                                     start=(c == 0), stop=(c == 3))
                st_sl = stage.rearrange("p (b t) -> p b t", b=BD)[
                    :, hb0:hb0 + HB, r0 - 1:r0 - 1 + nrows]
                nc.vector.tensor_scalar_add(
                    st_sl, P.rearrange("p (b t) -> p b t", b=HB), pjb)

            # ---- step 0: bias0 + w_hh matmuls on h0 (in-region bias-only)
            for X in (0, 1):
                bank_mms(X, RZ[X], ("r", "z"), bt0, wst0, 0)
                bank_mms(X, NI[X], ("hn", "in"), bt0, wst, 0,
                         wgates=("hn",))

            next_proj = [1, 1]
            dma_done = 0
            DMA_CHUNK = 50
            for t in range(steps):
                for X in (0, 1):
                    hb0 = X * HB
                    rz_s = wk.tile([128, 128], F32, tag=f"rz{X}")
                    a_s = wk.tile([128, 64], F32, tag=f"a{X}")
                    b_s = wk.tile([128, 64], F32, tag=f"b{X}")
                    n16 = wk.tile([128, 64], F16, tag=f"n{X}")
                    t1 = wk.tile([128, 4, HB], F32, tag=f"t1{X}")
                    u32 = wk.tile([128, 4, HB], F32, tag=f"u{X}")
                    e32 = wk.tile([128, 4, HB], F32, tag=f"e{X}")
                    h_t = hist[:, :, t, hb0:hb0 + HB]

                    # chain for step t (banks were filled by iteration t-1)
                    nc.scalar.activation(rz_s, RZ[X][:, 0:128], SIG)
                    z1m4 = rz_s[:, 64:128].rearrange("p (q b) -> p q b", q=4)
                    # u path on Pool: u32 = h - z1m*h = z*h, kept f32
                    nc.gpsimd.tensor_mul(t1, z1m4, h_t)
                    nc.gpsimd.tensor_sub(u32, h_t, t1)
                    # e path on DVE/Act: a = r*g_hn, b = a+g_in, n = tanh,
                    # e32 = z1m*n, then the join rides the same DVE queue
                    nc.vector.tensor_mul(a_s, rz_s[:, 0:64], NI[X][:, 0:64])
                    nc.vector.tensor_add(b_s, a_s, NI[X][:, 64:128])
                    nc.scalar.activation(n16, b_s, TANH)
                    n4 = n16.rearrange("p (q b) -> p q b", q=4)
                    nc.vector.tensor_mul(e32, z1m4, n4)
                    # join: h(t+1) = u32 + e32, single fp16 rounding
                    nc.vector.tensor_add(hist[:, :, t + 1, hb0:hb0 + HB],
                                         u32, e32)

                    # burst for G(t+1) on the joined hidden state
                    if t + 1 < steps:
                        bank_mms(X, RZ[X], ("r", "z"), bt, wst, t + 1)
                        bank_mms(X, NI[X], ("hn", "in"), bt, wst, t + 1)

                    if next_proj[X] + PROJ_EVERY <= t:
                        proj_rows(X, next_proj[X], PROJ_EVERY)
                        next_proj[X] += PROJ_EVERY

                # stream finalized output chunks once BOTH halves are done
                while dma_done + DMA_CHUNK < min(next_proj):
                    lo = dma_done
                    nc.sync.dma_start(
                        out=out_d[:, :, lo:lo + DMA_CHUNK].rearrange(
                            "b v t -> v b t"),
                        in_=stage.rearrange("p (b t) -> p b t", b=BD)[
                            :, :, lo:lo + DMA_CHUNK])
                    dma_done += DMA_CHUNK

            for X in (0, 1):
                while next_proj[X] <= steps:
                    nrows = min(PROJ_EVERY, steps + 1 - next_proj[X])
                    proj_rows(X, next_proj[X], nrows)
                    next_proj[X] += nrows

        if dma_done < steps:
            nc.sync.dma_start(
                out=out_d[:, :, dma_done:].rearrange("b v t -> v b t"),
                in_=stage.rearrange("p (b t) -> p b t", b=BD)[:, :, dma_done:])
    nc.compile()
    return nc


_CACHE = {}


def _get_nc(steps: int):
    if steps not in _CACHE:
        _CACHE[steps] = _build(steps)
    return _CACHE[steps]


def _prep_inputs(feat, embed_table, w_ih, w_hh, b_ih, b_hh, proj_w, proj_b):
    f32 = np.float32
    f16 = np.float16
    w_ih = np.asarray(w_ih, f32)
    w_hh = np.asarray(w_hh, f32)
    b_ih = np.asarray(b_ih, f32)
    b_hh = np.asarray(b_hh, f32)
    # fused gate weights, gate-major order r, z, hn, in
    # z gate negated: sigmoid(z psum) then directly equals 1 - z
    Wc = np.concatenate([w_ih[:H] + w_hh[:H],
                         -(w_ih[H:2 * H] + w_hh[H:2 * H]),
                         w_hh[2 * H:],
                         w_ih[2 * H:]], 0)          # [4H, H]
    bc = np.concatenate([b_ih[:H] + b_hh[:H],
                         -(b_ih[H:2 * H] + b_hh[H:2 * H]),
                         b_hh[2 * H:],
                         b_ih[2 * H:]], 0)          # [4H]

    x0 = np.asarray(embed_table, f32)[0]
    gi0 = w_ih @ x0 + b_ih                          # [3H]
    bc0 = np.concatenate([gi0[:H] + b_hh[:H],
                          -(gi0[H:2 * H] + b_hh[H:2 * H]),
                          b_hh[2 * H:],
                          gi0[2 * H:]], 0)          # [4H]
    W0 = np.concatenate([w_hh[:H], -w_hh[H:2 * H]], 0)  # [2H, H] r,z step-0

    # stationary blocks: wst[kp, ((g*4+q)*4+c)*128 + m] = Wc[g*512+q*128+m,
    #                                                        c*128+kp]
    wst = np.empty((128, 64 * 128), f32)
    for g in range(4):
        for q in range(4):
            for c in range(4):
                blk = ((g * 4 + q) * 4 + c) * 128
                wst[:, blk:blk + 128] = Wc[g * 512 + q * 128:
                                           g * 512 + (q + 1) * 128,
                                           c * 128:(c + 1) * 128].T
    wst0 = np.empty((128, 32 * 128), f32)
    for g in range(2):
        for q in range(4):
            for c in range(4):
                blk = ((g * 4 + q) * 4 + c) * 128
                wst0[:, blk:blk + 128] = W0[g * 512 + q * 128:
                                            g * 512 + (q + 1) * 128,
                                            c * 128:(c + 1) * 128].T

    proj_w = np.asarray(proj_w, f32)                # [V, H]
    pjt = np.empty((128, 4 * VOCAB), f32)
    for c in range(4):
        pjt[:, c * VOCAB:(c + 1) * VOCAB] = proj_w[:, c * 128:(c + 1) * 128].T

    feat = np.asarray(feat, f32)
    common = {
        "wst": wst.astype(f16),
        "wst0": wst0.astype(f16),
        "bt": bc.reshape(1, 2048).astype(f16),
        "bt0": bc0.reshape(1, 2048).astype(f16),
        "ones": np.ones((1, BD), f16),
        "pjt": pjt.astype(f16),
        "pjb": np.asarray(proj_b, f32).reshape(VOCAB, 1),
    }
    maps = []
    for i in range(NCORES):
        fs = feat[i * BD:(i + 1) * BD]              # [BD, H]
        h0g = np.ascontiguousarray(
            fs.T.reshape(4, 128, BD).transpose(1, 0, 2).reshape(128, 128))
        maps.append(dict(common, h0=h0g.astype(f16)))
    return maps


def kernel(feat, embed_table, w_ih, w_hh, b_ih, b_hh, proj_w, proj_b,
           _trace=False):
    nc = _get_nc(STEPS)
    in_maps = _prep_inputs(feat, embed_table, w_ih, w_hh, b_ih, b_hh,
                           proj_w, proj_b)
    res = run_bass_kernel_spmd(nc, in_maps, list(range(NCORES)), trace=_trace)
    out = np.concatenate([res.results[i]["out"] for i in range(NCORES)], 0)
    if _trace:
        kernel.last_exec_time_ns = res.exec_time_ns
        kernel.last_results = res
    return out


# revision 11
# speedup vs baseline: 1.2744x; 1.0258x over previous
"""Trainium2 Bass kernel for nn_CaptionModel (GRU caption decoder).

Model: h0 = feat; x0 = embed[<SOS>]; 200 GRU steps where the output hidden
state is fed back as the next input (x_t = h_t for t >= 1), then a linear
projection of every hidden state to vocab logits, output [B, V, T].

Since x_t == h_t for t >= 1 the two GRU matmuls fuse into one 2048-wide
gate matmul G = h @ Wc.T + bc with Wc = [w_ih_r+w_hh_r; -(w_ih_z+w_hh_z);
w_hh_n; w_ih_n] (z negated so sig(psum) = 1-z), gates r = sig(G0),
z1m = sig(G1) = 1-z, n = tanh(G3 + r*G2), h' = (h - z1m*h) + z1m*n.

KEY STRUCTURE (vs the single-recurrence version): the GRU recurrence is
latency-bound -- the serial loop [gate mms -> sigmoid -> a -> b -> tanh ->
e-mul -> join] crosses engines ~6 times at ~300ns per handoff, so one
recurrence cycles at ~3us regardless of batch.  Since the per-core batch
of 32 is data-parallel, we run TWO independent 16-sample recurrences
phase-shifted by half a cycle, interleaved on the same engines (each
engine is <60% busy).  Throughput doubles: ~1.5us/step effective.

Per half: gate PSUM is 2 single banks (RZ: r|z1m, NI: hn|in) -- PSUM
dependency tracking is bank-granular, so each bank is fully written
(bias first, start=True zeroes it) then read.  The PE consumes the
JOINED hidden h(t+1) = u32 + e32 (64 mms of 16 rows instead of 128) to
keep PE load at ~1us/step for both halves; u32 = h - z1m*h (Pool, f32)
and e32 = z1m*n (DVE, f32) are f32 so h' rounds to fp16 once per step
(fewer roundings than the u16/e16 split).  tanh chain on Act/DVE.

Sharding: pure data parallelism, batch 256 -> 32 per core on 8 cores,
weights replicated.
"""

import os
from contextlib import ExitStack

import numpy as np

import concourse.bass as bass
import concourse.tile as tile
from concourse import bacc, mybir
from concourse.bass_utils import run_bass_kernel_spmd

B, H, VOCAB = 256, 512, 100
STEPS = int(os.environ.get("KERNEL_STEPS", "200"))
NCORES = 8
BD = B // NCORES  # 32
HB = BD // 2      # 16 per interleaved half-recurrence
F16 = mybir.dt.float16
F32 = mybir.dt.float32
SIG = mybir.ActivationFunctionType.Sigmoid
TANH = mybir.ActivationFunctionType.Tanh

# gate order in the stationary weight blocks: r z hn in
GATES = ("r", "z", "hn", "in")
GI = {g: i for i, g in enumerate(GATES)}
PROJ_EVERY = 4


def _blk(g, q, c):
    return ((GI[g] * 4 + q) * 4 + c) * 128


def _colof(g, q):
    # within-bank columns: r/z/hn own their bank's cols 0:64 (q*HB);
    # in shares the NI bank at 64+q*HB
    return (64 if g == "in" else 0) + q * HB


def _build(steps: int):
    nc = bacc.Bacc("TRN2", target_bir_lowering=False, debug=False,
                   num_devices=NCORES)
    T1 = steps + 1

    wst_d = nc.dram_tensor("wst", [128, 64 * 128], F16, kind="ExternalInput").ap()
    wst0_d = nc.dram_tensor("wst0", [128, 32 * 128], F16, kind="ExternalInput").ap()
    h0_d = nc.dram_tensor("h0", [128, 128], F16, kind="ExternalInput").ap()
    bt_d = nc.dram_tensor("bt", [1, 2048], F16, kind="ExternalInput").ap()
    bt0_d = nc.dram_tensor("bt0", [1, 2048], F16, kind="ExternalInput").ap()
    ones_d = nc.dram_tensor("ones", [1, BD], F16, kind="ExternalInput").ap()
    pjt_d = nc.dram_tensor("pjt", [128, 4 * VOCAB], F16, kind="ExternalInput").ap()
    pjb_d = nc.dram_tensor("pjb", [VOCAB, 1], F32, kind="ExternalInput").ap()
    out_d = nc.dram_tensor("out", [BD, VOCAB, steps], F32,
                           kind="ExternalOutput").ap()

    with tile.TileContext(nc) as tc, ExitStack() as ctx:
        sg = ctx.enter_context(tc.tile_pool(name="sg", bufs=1))
        wk = ctx.enter_context(tc.tile_pool(name="wk", bufs=3))

        wst = sg.tile([128, 64 * 128], F16)
        nc.sync.dma_start(out=wst, in_=wst_d)
        wst0 = sg.tile([128, 32 * 128], F16)
        nc.sync.dma_start(out=wst0, in_=wst0_d)
        bt = sg.tile([1, 2048], F16)
        nc.sync.dma_start(out=bt, in_=bt_d)
        bt0 = sg.tile([1, 2048], F16)
        nc.sync.dma_start(out=bt0, in_=bt0_d)
        ones = sg.tile([1, BD], F16)
        nc.sync.dma_start(out=ones, in_=ones_d)
        pjt = sg.tile([128, 4 * VOCAB], F16)
        nc.sync.dma_start(out=pjt, in_=pjt_d)
        pjb = sg.tile([VOCAB, 1], F32)
        nc.sync.dma_start(out=pjb, in_=pjb_d)
        hist = sg.tile([128, 4, T1, BD], F16, name="hist")
        nc.sync.dma_start(out=hist[:, :, 0, :],
                          in_=h0_d.rearrange("p (q b) -> p q b", q=4))
        stage = sg.tile([VOCAB, BD * steps], F32, name="stage")

        with tc.tile_pool(name="gps", bufs=1, space="PSUM") as gpool, \
             tc.tile_pool(name="pps", bufs=2, space="PSUM") as ppool:
            # three single banks per half: R, Z, NI (hn|in).  Bank-granular
            # deps + whole-bank zeroing mean each accumulation group owns a
            # full 2KB bank; 6 gate banks + 2 proj banks = 8 exactly.
            # Separate R/Z banks let sig_r fire after only the 16 e_r mms.
            Rb = [gpool.tile([128, 512], F32, tag=f"R{i}", name=f"R{i}")
                  for i in range(2)]
            Zb = [gpool.tile([128, 512], F32, tag=f"Z{i}", name=f"Z{i}")
                  for i in range(2)]
            NI = [gpool.tile([128, 512], F32, tag=f"NI{i}", name=f"NI{i}")
                  for i in range(2)]

            def bank_of(X, g):
                return Rb[X] if g == "r" else (Zb[X] if g == "z" else NI[X])

            def bias_u_mms(X, gates, bias_src, w_src, rhs4, wgates=None):
                # first fill phase of each bank: bias mms (first carries
                # start=True, zeroing the bank) then the early-operand
                # weight mms.  wgates limits which gates get weight mms.
                hb0 = X * HB
                if wgates is None:
                    wgates = gates
                firsts = set()
                for g in gates:
                    bank = bank_of(X, g)
                    for q in range(4):
                        col = _colof(g, q)
                        first = id(bank) not in firsts
                        firsts.add(id(bank))
                        nc.tensor.matmul(
                            bank[:, col:col + HB],
                            bias_src[:, GI[g] * 512 + q * 128:
                                     GI[g] * 512 + (q + 1) * 128],
                            ones[:, 0:HB], start=first, stop=False,
                            skip_group_check=True)
                for g in wgates:
                    bank = bank_of(X, g)
                    for q in range(4):
                        for c in range(4):
                            wt = wst0 if (w_src is wst0 and g in ("r", "z")) \
                                else wst
                            nc.tensor.matmul(
                                bank[:, _colof(g, q):_colof(g, q) + HB],
                                wt[:, _blk(g, q, c):_blk(g, q, c) + 128],
                                rhs4[c],
                                start=False,
                                stop=(w_src is wst0 and q == 3 and c == 3),
                                skip_group_check=True)

            def e_mms(X, gates, rhs4, stop_gates):
                # second fill phase: the late-operand weight mms; gates in
                # stop_gates close their bank's group on their last mm
                for g in gates:
                    bank = bank_of(X, g)
                    for q in range(4):
                        for c in range(4):
                            stop = (g in stop_gates and q == 3 and c == 3)
                            nc.tensor.matmul(
                                bank[:, _colof(g, q):_colof(g, q) + HB],
                                wst[:, _blk(g, q, c):_blk(g, q, c) + 128],
                                rhs4[c],
                                start=False, stop=stop,
                                skip_group_check=True)

            def proj_rows(X, r0, nrows):
                hb0 = X * HB
                Pfull = ppool.tile([VOCAB, 512], F32, tag="P", name="Pfull")
                P = Pfull[:, 0:HB * nrows]
                for c in range(4):
                    rhs = hist[:, c, r0:r0 + nrows, hb0:hb0 + HB].rearrange(
                        "p t b -> p b t")
                    nc.tensor.matmul(P, pjt[:, c * VOCAB:(c + 1) * VOCAB], rhs,
                                     start=(c == 0), stop=(c == 3))
                st_sl = stage.rearrange("p (b t) -> p b t", b=BD)[
                    :, hb0:hb0 + HB, r0 - 1:r0 - 1 + nrows]
                nc.vector.tensor_scalar_add(
                    st_sl, P.rearrange("p (b t) -> p b t", b=HB), pjb)

            # ---- step 0: bias0 + w_hh matmuls on h0 (in-region bias-only)
            for X in (0, 1):
                h0c = [hist[:, c, 0, X * HB:X * HB + HB] for c in range(4)]
                bias_u_mms(X, ("r", "z", "hn", "in"), bt0, wst0, h0c,
                           wgates=("r", "z", "hn"))

            next_proj = [1, 1]
            dma_done = 0
            DMA_CHUNK = 50
            for t in range(steps):
                for X in (0, 1):
                    hb0 = X * HB
                    rz_s = wk.tile([128, 128], F32, tag=f"rz{X}")
                    a_s = wk.tile([128, 64], F32, tag=f"a{X}")
                    b_s = wk.tile([128, 64], F32, tag=f"b{X}")
                    n16 = wk.tile([128, 64], F16, tag=f"n{X}")
                    t1 = wk.tile([128, 4, HB], F32, tag=f"t1{X}")
                    u16 = wk.tile([128, 4, HB], F16, tag=f"u{X}")
                    e16 = wk.tile([128, 4, HB], F16, tag=f"e{X}")
                    h_t = hist[:, :, t, hb0:hb0 + HB]

                    # chain for step t (banks were filled by iteration t-1).
                    # critical loop: e_r mms -> sig_r -> a -> b -> tanh ->
                    # e16 -> (next burst's e mms).  The z/u path (sig_z ->
                    # t1 -> u on Pool) runs in parallel off-loop.
                    nc.scalar.activation(rz_s[:, 0:64], Rb[X][:, 0:64], SIG)
                    nc.scalar.activation(rz_s[:, 64:128], Zb[X][:, 0:64],
                                         SIG)
                    z1m4 = rz_s[:, 64:128].rearrange("p (q b) -> p q b", q=4)
                    # u path on Pool: t1 = (1-z)*h in f32, u = h - t1
                    nc.gpsimd.tensor_mul(t1, z1m4, h_t)
                    nc.gpsimd.tensor_sub(u16, h_t, t1)
                    # e path on DVE/Act
                    nc.vector.tensor_mul(a_s, rz_s[:, 0:64], NI[X][:, 0:64])
                    nc.vector.tensor_add(b_s, a_s, NI[X][:, 64:128])
                    nc.scalar.activation(n16, b_s, TANH)
                    n4 = n16.rearrange("p (q b) -> p q b", q=4)
                    nc.vector.tensor_mul(e16, z1m4, n4)
                    # h join for hist/proj (off the critical loop, on Pool)
                    nc.gpsimd.tensor_add(hist[:, :, t + 1, hb0:hb0 + HB],
                                         u16, e16)

                    # burst for G(t+1): bias + u-operand mms first (u is
                    # ready early via the Pool path), then the e-operand
                    # mms in r -> z -> hn/in order so sig_r(t+1) fires
                    # after only 16 e_r mms
                    if t + 1 < steps:
                        u4 = [u16[:, c, :] for c in range(4)]
                        e4 = [e16[:, c, :] for c in range(4)]
                        bias_u_mms(X, ("r", "z", "hn", "in"), bt, wst, u4)
                        e_mms(X, ("r",), e4, ("r",))
                        e_mms(X, ("z",), e4, ("z",))
                        e_mms(X, ("hn", "in"), e4, ("in",))

                    if next_proj[X] + PROJ_EVERY <= t:
                        proj_rows(X, next_proj[X], PROJ_EVERY)
                        next_proj[X] += PROJ_EVERY

                # stream finalized output chunks once BOTH halves are done
                while dma_done + DMA_CHUNK < min(next_proj):
                    lo = dma_done
                    nc.sync.dma_start(
                        out=out_d[:, :, lo:lo + DMA_CHUNK].rearrange(
                            "b v t -> v b t"),
                        in_=stage.rearrange("p (b t) -> p b t", b=BD)[
                            :, :, lo:lo + DMA_CHUNK])
                    dma_done += DMA_CHUNK

            for X in (0, 1):
                while next_proj[X] <= steps:
                    nrows = min(PROJ_EVERY, steps + 1 - next_proj[X])
                    proj_rows(X, next_proj[X], nrows)
                    next_proj[X] += nrows

        if dma_done < steps:
            nc.sync.dma_start(
                out=out_d[:, :, dma_done:].rearrange("b v t -> v b t"),
                in_=stage.rearrange("p (b t) -> p b t", b=BD)[:, :, dma_done:])
    nc.compile()
    return nc


_CACHE = {}


def _get_nc(steps: int):
    if steps not in _CACHE:
        _CACHE[steps] = _build(steps)
    return _CACHE[steps]


def _prep_inputs(feat, embed_table, w_ih, w_hh, b_ih, b_hh, proj_w, proj_b):
    f32 = np.float32
    f16 = np.float16
    w_ih = np.asarray(w_ih, f32)
    w_hh = np.asarray(w_hh, f32)
    b_ih = np.asarray(b_ih, f32)
    b_hh = np.asarray(b_hh, f32)
    # fused gate weights, gate-major order r, z, hn, in
    # z gate negated: sigmoid(z psum) then directly equals 1 - z
    Wc = np.concatenate([w_ih[:H] + w_hh[:H],
                         -(w_ih[H:2 * H] + w_hh[H:2 * H]),
                         w_hh[2 * H:],
                         w_ih[2 * H:]], 0)          # [4H, H]
    bc = np.concatenate([b_ih[:H] + b_hh[:H],
                         -(b_ih[H:2 * H] + b_hh[H:2 * H]),
                         b_hh[2 * H:],
                         b_ih[2 * H:]], 0)          # [4H]

    x0 = np.asarray(embed_table, f32)[0]
    gi0 = w_ih @ x0 + b_ih                          # [3H]
    bc0 = np.concatenate([gi0[:H] + b_hh[:H],
                          -(gi0[H:2 * H] + b_hh[H:2 * H]),
                          b_hh[2 * H:],
                          gi0[2 * H:]], 0)          # [4H]
    W0 = np.concatenate([w_hh[:H], -w_hh[H:2 * H]], 0)  # [2H, H] r,z step-0

    # stationary blocks: wst[kp, ((g*4+q)*4+c)*128 + m] = Wc[g*512+q*128+m,
    #                                                        c*128+kp]
    wst = np.empty((128, 64 * 128), f32)
    for g in range(4):
        for q in range(4):
            for c in range(4):
                blk = ((g * 4 + q) * 4 + c) * 128
                wst[:, blk:blk + 128] = Wc[g * 512 + q * 128:
                                           g * 512 + (q + 1) * 128,
                                           c * 128:(c + 1) * 128].T
    wst0 = np.empty((128, 32 * 128), f32)
    for g in range(2):
        for q in range(4):
            for c in range(4):
                blk = ((g * 4 + q) * 4 + c) * 128
                wst0[:, blk:blk + 128] = W0[g * 512 + q * 128:
                                            g * 512 + (q + 1) * 128,
                                            c * 128:(c + 1) * 128].T

    proj_w = np.asarray(proj_w, f32)                # [V, H]
    pjt = np.empty((128, 4 * VOCAB), f32)
    for c in range(4):
        pjt[:, c * VOCAB:(c + 1) * VOCAB] = proj_w[:, c * 128:(c + 1) * 128].T

    feat = np.asarray(feat, f32)
    common = {
        "wst": wst.astype(f16),
        "wst0": wst0.astype(f16),
        "bt": bc.reshape(1, 2048).astype(f16),
        "bt0": bc0.reshape(1, 2048).astype(f16),
        "ones": np.ones((1, BD), f16),
        "pjt": pjt.astype(f16),
        "pjb": np.asarray(proj_b, f32).reshape(VOCAB, 1),
    }
    maps = []
    for i in range(NCORES):
        fs = feat[i * BD:(i + 1) * BD]              # [BD, H]
        h0g = np.ascontiguousarray(
            fs.T.reshape(4, 128, BD).transpose(1, 0, 2).reshape(128, 128))
        maps.append(dict(common, h0=h0g.astype(f16)))
    return maps


def kernel(feat, embed_table, w_ih, w_hh, b_ih, b_hh, proj_w, proj_b,
           _trace=False):
    nc = _get_nc(STEPS)
    in_maps = _prep_inputs(feat, embed_table, w_ih, w_hh, b_ih, b_hh,
                           proj_w, proj_b)
    res = run_bass_kernel_spmd(nc, in_maps, list(range(NCORES)), trace=_trace)
    out = np.concatenate([res.results[i]["out"] for i in range(NCORES)], 0)
    if _trace:
        kernel.last_exec_time_ns = res.exec_time_ns
        kernel.last_results = res
    return out


# revision 21
# speedup vs baseline: 1.2898x; 1.0121x over previous
"""Trainium2 Bass kernel for nn_CaptionModel (GRU caption decoder).

Model: h0 = feat; x0 = embed[<SOS>]; 200 GRU steps where the output hidden
state is fed back as the next input (x_t = h_t for t >= 1), then a linear
projection of every hidden state to vocab logits, output [B, V, T].

Since x_t == h_t for t >= 1 the two GRU matmuls fuse into one 2048-wide
gate matmul G = h @ Wc.T + bc with Wc = [w_ih_r+w_hh_r; -(w_ih_z+w_hh_z);
w_hh_n; w_ih_n] (z negated so sig(psum) = 1-z), gates r = sig(G0),
z1m = sig(G1) = 1-z, n = tanh(G3 + r*G2), h' = (h - z1m*h) + z1m*n.

KEY STRUCTURE (vs the single-recurrence version): the GRU recurrence is
latency-bound -- the serial loop [gate mms -> sigmoid -> a -> b -> tanh ->
e-mul -> join] crosses engines ~6 times at ~300ns per handoff, so one
recurrence cycles at ~3us regardless of batch.  Since the per-core batch
of 32 is data-parallel, we run TWO independent 16-sample recurrences
phase-shifted by half a cycle, interleaved on the same engines (each
engine is <60% busy).  Throughput doubles: ~1.5us/step effective.

Per half: gate PSUM is 2 single banks (RZ: r|z1m, NI: hn|in) -- PSUM
dependency tracking is bank-granular, so each bank is fully written
(bias first, start=True zeroes it) then read.  The PE consumes the
JOINED hidden h(t+1) = u32 + e32 (64 mms of 16 rows instead of 128) to
keep PE load at ~1us/step for both halves; u32 = h - z1m*h (Pool, f32)
and e32 = z1m*n (DVE, f32) are f32 so h' rounds to fp16 once per step
(fewer roundings than the u16/e16 split).  tanh chain on Act/DVE.

Sharding: pure data parallelism, batch 256 -> 32 per core on 8 cores,
weights replicated.
"""

import os
from contextlib import ExitStack

import numpy as np

import concourse.bass as bass
import concourse.tile as tile
from concourse import bacc, mybir
from concourse.bass_utils import run_bass_kernel_spmd

B, H, VOCAB = 256, 512, 100
STEPS = int(os.environ.get("KERNEL_STEPS", "200"))
NCORES = 8
BD = B // NCORES  # 32
HB = BD // 2      # 16 per interleaved half-recurrence
F16 = mybir.dt.float16
F32 = mybir.dt.float32
SIG = mybir.ActivationFunctionType.Sigmoid
TANH = mybir.ActivationFunctionType.Tanh

# gate order in the stationary weight blocks: r z hn in
GATES = ("r", "z", "hn", "in")
GI = {g: i for i, g in enumerate(GATES)}
PROJ_EVERY = 4


def _blk(g, q, c):
    return ((GI[g] * 4 + q) * 4 + c) * 128


def _colof(g, q):
    # within-bank columns: r/hn at q*HB (cols 0:64), z/in at 64+q*HB
    return (64 if g in ("z", "in") else 0) + q * HB


def _build(steps: int):
    nc = bacc.Bacc("TRN2", target_bir_lowering=False, debug=False,
                   num_devices=NCORES)
    T1 = steps + 1

    wst_d = nc.dram_tensor("wst", [128, 64 * 128], F16, kind="ExternalInput").ap()
    wst0_d = nc.dram_tensor("wst0", [128, 32 * 128], F16, kind="ExternalInput").ap()
    h0_d = nc.dram_tensor("h0", [128, 128], F16, kind="ExternalInput").ap()
    bt_d = nc.dram_tensor("bt", [1, 2048], F16, kind="ExternalInput").ap()
    bt0_d = nc.dram_tensor("bt0", [1, 2048], F16, kind="ExternalInput").ap()
    ones_d = nc.dram_tensor("ones", [1, BD], F16, kind="ExternalInput").ap()
    pjt_d = nc.dram_tensor("pjt", [128, 4 * VOCAB], F16, kind="ExternalInput").ap()
    pjb_d = nc.dram_tensor("pjb", [VOCAB, 1], F32, kind="ExternalInput").ap()
    out_d = nc.dram_tensor("out", [BD, VOCAB, steps], F32,
                           kind="ExternalOutput").ap()

    with tile.TileContext(nc) as tc, ExitStack() as ctx:
        sg = ctx.enter_context(tc.tile_pool(name="sg", bufs=1))
        wk = ctx.enter_context(tc.tile_pool(name="wk", bufs=3))

        wst = sg.tile([128, 64 * 128], F16)
        nc.sync.dma_start(out=wst, in_=wst_d)
        wst0 = sg.tile([128, 32 * 128], F16)
        nc.sync.dma_start(out=wst0, in_=wst0_d)
        bt = sg.tile([1, 2048], F16)
        nc.sync.dma_start(out=bt, in_=bt_d)
        bt0 = sg.tile([1, 2048], F16)
        nc.sync.dma_start(out=bt0, in_=bt0_d)
        ones = sg.tile([1, BD], F16)
        nc.sync.dma_start(out=ones, in_=ones_d)
        pjt = sg.tile([128, 4 * VOCAB], F16)
        nc.sync.dma_start(out=pjt, in_=pjt_d)
        pjb = sg.tile([VOCAB, 1], F32)
        nc.sync.dma_start(out=pjb, in_=pjb_d)
        hist = sg.tile([128, 4, T1, BD], F16, name="hist")
        nc.sync.dma_start(out=hist[:, :, 0, :],
                          in_=h0_d.rearrange("p (q b) -> p q b", q=4))
        stage = sg.tile([VOCAB, BD * steps], F32, name="stage")

        with tc.tile_pool(name="gps", bufs=1, space="PSUM") as gpool, \
             tc.tile_pool(name="pps", bufs=2, space="PSUM") as ppool:
            # two single banks per half: RZ (r|z) and NI (hn|in).  Bank-
            # granular deps + whole-bank zeroing mean each accumulation
            # group owns a full 2KB bank; 4 gate banks + 2 proj banks = 6.
            # One combined sigmoid over [r|z1m] feeds both the e-chain (r)
            # and the u-path (z1m) in a single Act op.
            RZ = [gpool.tile([128, 512], F32, tag=f"RZ{i}", name=f"RZ{i}")
                  for i in range(2)]
            NI = [gpool.tile([128, 512], F32, tag=f"NI{i}", name=f"NI{i}")
                  for i in range(2)]

            def bank_of(X, g):
                return RZ[X] if g in ("r", "z") else NI[X]

            def bias_u_mms(X, gates, bias_src, w_src, rhs4, wgates=None):
                # first fill phase of each bank: bias mms (first carries
                # start=True, zeroing the bank) then the early-operand
                # weight mms.  wgates limits which gates get weight mms.
                hb0 = X * HB
                if wgates is None:
                    wgates = gates
                firsts = set()
                for g in gates:
                    bank = bank_of(X, g)
                    for q in range(4):
                        col = _colof(g, q)
                        first = id(bank) not in firsts
                        firsts.add(id(bank))
                        nc.tensor.matmul(
                            bank[:, col:col + HB],
                            bias_src[:, GI[g] * 512 + q * 128:
                                     GI[g] * 512 + (q + 1) * 128],
                            ones[:, 0:HB], start=first, stop=False,
                            skip_group_check=True)
                for g in wgates:
                    bank = bank_of(X, g)
                    for q in range(4):
                        for c in range(4):
                            wt = wst0 if (w_src is wst0 and g in ("r", "z")) \
                                else wst
                            nc.tensor.matmul(
                                bank[:, _colof(g, q):_colof(g, q) + HB],
                                wt[:, _blk(g, q, c):_blk(g, q, c) + 128],
                                rhs4[c],
                                start=False,
                                stop=(w_src is wst0 and q == 3 and c == 3),
                                skip_group_check=True)

            def e_mms(X, gates, rhs4, stop_gates, cs=(0, 1, 2, 3)):
                # second fill phase: the late-operand weight mms; gates in
                # stop_gates close their bank's group on their last mm
                for g in gates:
                    bank = bank_of(X, g)
                    for q in range(4):
                        for c in cs:
                            stop = (g in stop_gates and q == 3 and c == cs[-1])
                            nc.tensor.matmul(
                                bank[:, _colof(g, q):_colof(g, q) + HB],
                                wst[:, _blk(g, q, c):_blk(g, q, c) + 128],
                                rhs4[c],
                                start=False, stop=stop,
                                skip_group_check=True)

            def proj_rows(X, r0, nrows):
                hb0 = X * HB
                Pfull = ppool.tile([VOCAB, 512], F32, tag="P", name="Pfull")
                P = Pfull[:, 0:HB * nrows]
                for c in range(4):
                    rhs = hist[:, c, r0:r0 + nrows, hb0:hb0 + HB].rearrange(
                        "p t b -> p b t")
                    nc.tensor.matmul(P, pjt[:, c * VOCAB:(c + 1) * VOCAB], rhs,
                                     start=(c == 0), stop=(c == 3))
                st_sl = stage.rearrange("p (b t) -> p b t", b=BD)[
                    :, hb0:hb0 + HB, r0 - 1:r0 - 1 + nrows]
                nc.vector.tensor_scalar_add(
                    st_sl, P.rearrange("p (b t) -> p b t", b=HB), pjb)

            # ---- step 0: bias0 + w_hh matmuls on h0 (in-region bias-only)
            for X in (0, 1):
                h0c = [hist[:, c, 0, X * HB:X * HB + HB] for c in range(4)]
                bias_u_mms(X, ("r", "z", "hn", "in"), bt0, wst0, h0c,
                           wgates=("r", "z", "hn"))

            next_proj = [1, 1]
            dma_done = 0
            DMA_CHUNK = 50
            for t in range(steps):
                for X in (0, 1):
                    hb0 = X * HB
                    rz_s = wk.tile([128, 128], F32, tag=f"rz{X}")
                    a_s = wk.tile([128, 64], F32, tag=f"a{X}")
                    b_s = wk.tile([128, 64], F32, tag=f"b{X}")
                    n16 = wk.tile([128, 64], F16, tag=f"n{X}")
                    # t1 in f16 keeps the u-path math bitwise identical to the
                    # validated single-recurrence kernel (final max-err of this
                    # 200-step feedback loop is sensitive to rounding placement)
                    t1 = wk.tile([128, 4, HB], F16, tag=f"t1{X}")
                    u16 = wk.tile([128, 4, HB], F16, tag=f"u{X}")
                    e16 = wk.tile([128, 4, HB], F16, tag=f"e{X}")
                    h_t = hist[:, :, t, hb0:hb0 + HB]

                    # chain for step t (banks were filled by iteration t-1).
                    # critical loop: e_rz mms -> sig -> a -> b -> tanh ->
                    # e16 -> (next burst's e mms).  The u-path (t1 -> u on
                    # Pool, from z1m) runs in parallel off-loop.
                    nc.scalar.activation(rz_s, RZ[X][:, 0:128], SIG)
                    z1m4 = rz_s[:, 64:128].rearrange("p (q b) -> p q b", q=4)
                    # u path on Pool: t1 = (1-z)*h in f32, u = h - t1
                    nc.gpsimd.tensor_mul(t1, z1m4, h_t)
                    nc.gpsimd.tensor_sub(u16, h_t, t1)
                    # e path on DVE/Act; e16 in c-halves so the next
                    # burst's e_rz(c01) mms start early; the hist join
                    # rides the DVE queue after e16 (off the critical loop)
                    nc.vector.tensor_mul(a_s, rz_s[:, 0:64], NI[X][:, 0:64])
                    nc.vector.tensor_add(b_s, a_s, NI[X][:, 64:128])
                    nc.scalar.activation(n16, b_s, TANH)
                    n4 = n16.rearrange("p (q b) -> p q b", q=4)
                    nc.vector.tensor_mul(e16[:, 0:2, :], z1m4[:, 0:2, :],
                                         n4[:, 0:2, :])
                    nc.vector.tensor_mul(e16[:, 2:4, :], z1m4[:, 2:4, :],
                                         n4[:, 2:4, :])
                    nc.vector.tensor_add(hist[:, :, t + 1, hb0:hb0 + HB],
                                         u16, e16)

                    # burst for G(t+1): bias + u-operand mms first (u is
                    # ready early via the Pool path), then the e-operand
                    # mms rz-first (c01 then c23) so sig(t+1) fires after
                    # only the 32 e_rz mms
                    if t + 1 < steps:
                        u4 = [u16[:, c, :] for c in range(4)]
                        e4 = [e16[:, c, :] for c in range(4)]
                        bias_u_mms(X, ("r", "z", "hn", "in"), bt, wst, u4)
                        e_mms(X, ("r", "z"), e4, (), cs=(0, 1))
                        e_mms(X, ("r", "z"), e4, ("z",), cs=(2, 3))
                        e_mms(X, ("hn", "in"), e4, ("in",))

                    if next_proj[X] + PROJ_EVERY <= t:
                        proj_rows(X, next_proj[X], PROJ_EVERY)
                        next_proj[X] += PROJ_EVERY

                # stream finalized output chunks once BOTH halves are done
                while dma_done + DMA_CHUNK < min(next_proj):
                    lo = dma_done
                    nc.sync.dma_start(
                        out=out_d[:, :, lo:lo + DMA_CHUNK].rearrange(
                            "b v t -> v b t"),
                        in_=stage.rearrange("p (b t) -> p b t", b=BD)[
                            :, :, lo:lo + DMA_CHUNK])
                    dma_done += DMA_CHUNK

            for X in (0, 1):
                while next_proj[X] <= steps:
                    nrows = min(PROJ_EVERY, steps + 1 - next_proj[X])
                    proj_rows(X, next_proj[X], nrows)
                    next_proj[X] += nrows

        if dma_done < steps:
            nc.sync.dma_start(
                out=out_d[:, :, dma_done:].rearrange("b v t -> v b t"),
                in_=stage.rearrange("p (b t) -> p b t", b=BD)[:, :, dma_done:])
    nc.compile()
    return nc


_CACHE = {}


def _get_nc(steps: int):
    if steps not in _CACHE:
        _CACHE[steps] = _build(steps)
    return _CACHE[steps]


def _prep_inputs(feat, embed_table, w_ih, w_hh, b_ih, b_hh, proj_w, proj_b):
    f32 = np.float32
    f16 = np.float16
    w_ih = np.asarray(w_ih, f32)
    w_hh = np.asarray(w_hh, f32)
    b_ih = np.asarray(b_ih, f32)
    b_hh = np.asarray(b_hh, f32)
    # fused gate weights, gate-major order r, z, hn, in
    # z gate negated: sigmoid(z psum) then directly equals 1 - z
    Wc = np.concatenate([w_ih[:H] + w_hh[:H],
                         -(w_ih[H:2 * H] + w_hh[H:2 * H]),
                         w_hh[2 * H:],
                         w_ih[2 * H:]], 0)          # [4H, H]
    bc = np.concatenate([b_ih[:H] + b_hh[:H],
                         -(b_ih[H:2 * H] + b_hh[H:2 * H]),
                         b_hh[2 * H:],
                         b_ih[2 * H:]], 0)          # [4H]

    x0 = np.asarray(embed_table, f32)[0]
    gi0 = w_ih @ x0 + b_ih                          # [3H]
    bc0 = np.concatenate([gi0[:H] + b_hh[:H],
                          -(gi0[H:2 * H] + b_hh[H:2 * H]),
                          b_hh[2 * H:],
                          gi0[2 * H:]], 0)          # [4H]
    W0 = np.concatenate([w_hh[:H], -w_hh[H:2 * H]], 0)  # [2H, H] r,z step-0

    # stationary blocks: wst[kp, ((g*4+q)*4+c)*128 + m] = Wc[g*512+q*128+m,
    #                                                        c*128+kp]
    wst = np.empty((128, 64 * 128), f32)
    for g in range(4):
        for q in range(4):
            for c in range(4):
                blk = ((g * 4 + q) * 4 + c) * 128
                wst[:, blk:blk + 128] = Wc[g * 512 + q * 128:
                                           g * 512 + (q + 1) * 128,
                                           c * 128:(c + 1) * 128].T
    wst0 = np.empty((128, 32 * 128), f32)
    for g in range(2):
        for q in range(4):
            for c in range(4):
                blk = ((g * 4 + q) * 4 + c) * 128
                wst0[:, blk:blk + 128] = W0[g * 512 + q * 128:
                                            g * 512 + (q + 1) * 128,
                                            c * 128:(c + 1) * 128].T

    proj_w = np.asarray(proj_w, f32)                # [V, H]
    pjt = np.empty((128, 4 * VOCAB), f32)
    for c in range(4):
        pjt[:, c * VOCAB:(c + 1) * VOCAB] = proj_w[:, c * 128:(c + 1) * 128].T

    feat = np.asarray(feat, f32)
    common = {
        "wst": wst.astype(f16),
        "wst0": wst0.astype(f16),
        "bt": bc.reshape(1, 2048).astype(f16),
        "bt0": bc0.reshape(1, 2048).astype(f16),
        "ones": np.ones((1, BD), f16),
        "pjt": pjt.astype(f16),
        "pjb": np.asarray(proj_b, f32).reshape(VOCAB, 1),
    }
    maps = []
    for i in range(NCORES):
        fs = feat[i * BD:(i + 1) * BD]              # [BD, H]
        h0g = np.ascontiguousarray(
            fs.T.reshape(4, 128, BD).transpose(1, 0, 2).reshape(128, 128))
        maps.append(dict(common, h0=h0g.astype(f16)))
    return maps


def kernel(feat, embed_table, w_ih, w_hh, b_ih, b_hh, proj_w, proj_b,
           _trace=False):
    nc = _get_nc(STEPS)
    in_maps = _prep_inputs(feat, embed_table, w_ih, w_hh, b_ih, b_hh,
                           proj_w, proj_b)
    res = run_bass_kernel_spmd(nc, in_maps, list(range(NCORES)), trace=_trace)
    out = np.concatenate([res.results[i]["out"] for i in range(NCORES)], 0)
    if _trace:
        kernel.last_exec_time_ns = res.exec_time_ns
        kernel.last_results = res
    return out


# revision 24
# speedup vs baseline: 1.3472x; 1.0445x over previous
"""Trainium2 Bass kernel for nn_CaptionModel (GRU caption decoder).

Model: h0 = feat; x0 = embed[<SOS>]; 200 GRU steps where the output hidden
state is fed back as the next input (x_t = h_t for t >= 1), then a linear
projection of every hidden state to vocab logits, output [B, V, T].

Since x_t == h_t for t >= 1 the two GRU matmuls fuse into one 2048-wide
gate matmul G = h @ Wc.T + bc with Wc = [w_ih_r+w_hh_r; -(w_ih_z+w_hh_z);
w_hh_n; w_ih_n] (z negated so sig(psum) = 1-z), gates r = sig(G0),
z1m = sig(G1) = 1-z, n = tanh(G3 + r*G2), h' = (h - z1m*h) + z1m*n.

KEY STRUCTURE (vs the single-recurrence version): the GRU recurrence is
latency-bound -- the serial loop [gate mms -> sigmoid -> a -> b -> tanh ->
e-mul -> join] crosses engines ~6 times at ~300ns per handoff, so one
recurrence cycles at ~3us regardless of batch.  Since the per-core batch
of 32 is data-parallel, we run TWO independent 16-sample recurrences
phase-shifted by half a cycle, interleaved on the same engines (each
engine is <60% busy).  Throughput doubles: ~1.5us/step effective.

Per half: gate PSUM is 2 single banks (RZ: r|z1m, NI: hn|in) -- PSUM
dependency tracking is bank-granular, so each bank is fully written
(bias first, start=True zeroes it) then read.  The PE consumes the
JOINED hidden h(t+1) = u32 + e32 (64 mms of 16 rows instead of 128) to
keep PE load at ~1us/step for both halves; u32 = h - z1m*h (Pool, f32)
and e32 = z1m*n (DVE, f32) are f32 so h' rounds to fp16 once per step
(fewer roundings than the u16/e16 split).  tanh chain on Act/DVE.

Sharding: pure data parallelism, batch 256 -> 32 per core on 8 cores,
weights replicated.
"""

import os
from contextlib import ExitStack

import numpy as np

import concourse.bass as bass
import concourse.tile as tile
from concourse import bacc, mybir
from concourse.bass_utils import run_bass_kernel_spmd

B, H, VOCAB = 256, 512, 100
STEPS = int(os.environ.get("KERNEL_STEPS", "200"))
NCORES = 8
BD = B // NCORES  # 32
HB = BD // 2      # 16 per interleaved half-recurrence
F16 = mybir.dt.float16
F32 = mybir.dt.float32
SIG = mybir.ActivationFunctionType.Sigmoid
TANH = mybir.ActivationFunctionType.Tanh

# gate order in the stationary weight blocks: r z hn in
GATES = ("r", "z", "hn", "in")
GI = {g: i for i, g in enumerate(GATES)}
PROJ_EVERY = 8


def _blk(g, q, c):
    return ((GI[g] * 4 + q) * 4 + c) * 128


def _colof(g, q):
    # within-bank columns: r/hn at q*HB (cols 0:64), z/in at 64+q*HB
    return (64 if g in ("z", "in") else 0) + q * HB


def _build(steps: int):
    nc = bacc.Bacc("TRN2", target_bir_lowering=False, debug=False,
                   num_devices=NCORES)
    T1 = steps + 1

    wst_d = nc.dram_tensor("wst", [128, 64 * 128], F16, kind="ExternalInput").ap()
    wst0_d = nc.dram_tensor("wst0", [128, 32 * 128], F16, kind="ExternalInput").ap()
    h0_d = nc.dram_tensor("h0", [128, 128], F16, kind="ExternalInput").ap()
    bt_d = nc.dram_tensor("bt", [1, 2048], F16, kind="ExternalInput").ap()
    bt0_d = nc.dram_tensor("bt0", [1, 2048], F16, kind="ExternalInput").ap()
    ones_d = nc.dram_tensor("ones", [1, BD], F16, kind="ExternalInput").ap()
    pjt_d = nc.dram_tensor("pjt", [128, 4 * VOCAB], F16, kind="ExternalInput").ap()
    pjb_d = nc.dram_tensor("pjb", [VOCAB, 1], F32, kind="ExternalInput").ap()
    out_d = nc.dram_tensor("out", [BD, VOCAB, steps], F32,
                           kind="ExternalOutput").ap()

    with tile.TileContext(nc) as tc, ExitStack() as ctx:
        sg = ctx.enter_context(tc.tile_pool(name="sg", bufs=1))
        wk = ctx.enter_context(tc.tile_pool(name="wk", bufs=3))

        wst = sg.tile([128, 64 * 128], F16)
        nc.sync.dma_start(out=wst, in_=wst_d)
        wst0 = sg.tile([128, 32 * 128], F16)
        nc.sync.dma_start(out=wst0, in_=wst0_d)
        bt = sg.tile([1, 2048], F16)
        nc.sync.dma_start(out=bt, in_=bt_d)
        bt0 = sg.tile([1, 2048], F16)
        nc.sync.dma_start(out=bt0, in_=bt0_d)
        ones = sg.tile([1, BD], F16)
        nc.sync.dma_start(out=ones, in_=ones_d)
        pjt = sg.tile([128, 4 * VOCAB], F16)
        nc.sync.dma_start(out=pjt, in_=pjt_d)
        pjb = sg.tile([VOCAB, 1], F32)
        nc.sync.dma_start(out=pjb, in_=pjb_d)
        hist = sg.tile([128, 4, T1, BD], F16, name="hist")
        nc.sync.dma_start(out=hist[:, :, 0, :],
                          in_=h0_d.rearrange("p (q b) -> p q b", q=4))
        stage = sg.tile([VOCAB, BD * steps], F32, name="stage")

        with tc.tile_pool(name="gps", bufs=1, space="PSUM") as gpool, \
             tc.tile_pool(name="pps", bufs=2, space="PSUM") as ppool:
            # two single banks per half: RZ (r|z) and NI (hn|in).  Bank-
            # granular deps + whole-bank zeroing mean each accumulation
            # group owns a full 2KB bank; 4 gate banks + 2 proj banks = 6.
            # One combined sigmoid over [r|z1m] feeds both the e-chain (r)
            # and the u-path (z1m) in a single Act op.
            RZ = [gpool.tile([128, 512], F32, tag=f"RZ{i}", name=f"RZ{i}")
                  for i in range(2)]
            NI = [gpool.tile([128, 512], F32, tag=f"NI{i}", name=f"NI{i}")
                  for i in range(2)]

            def bank_of(X, g):
                return RZ[X] if g in ("r", "z") else NI[X]

            def bias_u_mms(X, gates, bias_src, w_src, rhs4, wgates=None):
                # first fill phase of each bank: bias mms (first carries
                # start=True, zeroing the bank) then the early-operand
                # weight mms.  wgates limits which gates get weight mms.
                hb0 = X * HB
                if wgates is None:
                    wgates = gates
                firsts = set()
                for g in gates:
                    bank = bank_of(X, g)
                    for q in range(4):
                        col = _colof(g, q)
                        first = id(bank) not in firsts
                        firsts.add(id(bank))
                        nc.tensor.matmul(
                            bank[:, col:col + HB],
                            bias_src[:, GI[g] * 512 + q * 128:
                                     GI[g] * 512 + (q + 1) * 128],
                            ones[:, 0:HB], start=first, stop=False,
                            skip_group_check=True)
                for g in wgates:
                    bank = bank_of(X, g)
                    for q in range(4):
                        for c in range(4):
                            wt = wst0 if (w_src is wst0 and g in ("r", "z")) \
                                else wst
                            nc.tensor.matmul(
                                bank[:, _colof(g, q):_colof(g, q) + HB],
                                wt[:, _blk(g, q, c):_blk(g, q, c) + 128],
                                rhs4[c],
                                start=False,
                                stop=(w_src is wst0 and q == 3 and c == 3),
                                skip_group_check=True)

            def e_mms(X, gates, rhs4, stop_gates, cs=(0, 1, 2, 3)):
                # second fill phase: the late-operand weight mms; gates in
                # stop_gates close their bank's group on their last mm
                for g in gates:
                    bank = bank_of(X, g)
                    for q in range(4):
                        for c in cs:
                            stop = (g in stop_gates and q == 3 and c == cs[-1])
                            nc.tensor.matmul(
                                bank[:, _colof(g, q):_colof(g, q) + HB],
                                wst[:, _blk(g, q, c):_blk(g, q, c) + 128],
                                rhs4[c],
                                start=False, stop=stop,
                                skip_group_check=True)

            def proj_rows(X, r0, nrows):
                hb0 = X * HB
                Pfull = ppool.tile([VOCAB, 512], F32, tag="P", name="Pfull")
                P = Pfull[:, 0:HB * nrows]
                for c in range(4):
                    rhs = hist[:, c, r0:r0 + nrows, hb0:hb0 + HB].rearrange(
                        "p t b -> p b t")
                    nc.tensor.matmul(P, pjt[:, c * VOCAB:(c + 1) * VOCAB], rhs,
                                     start=(c == 0), stop=(c == 3))
                st_sl = stage.rearrange("p (b t) -> p b t", b=BD)[
                    :, hb0:hb0 + HB, r0 - 1:r0 - 1 + nrows]
                nc.vector.tensor_scalar_add(
                    st_sl, P.rearrange("p (b t) -> p b t", b=HB), pjb)

            # ---- step 0: bias0 + w_hh matmuls on h0 (in-region bias-only)
            for X in (0, 1):
                h0c = [hist[:, c, 0, X * HB:X * HB + HB] for c in range(4)]
                bias_u_mms(X, ("r", "z", "hn", "in"), bt0, wst0, h0c,
                           wgates=("r", "z", "hn"))

            next_proj = [1, 1]
            dma_done = 0
            DMA_CHUNK = 50
            for t in range(steps):
                for X in (0, 1):
                    hb0 = X * HB
                    rz_s = wk.tile([128, 128], F32, tag=f"rz{X}")
                    a_s = wk.tile([128, 64], F32, tag=f"a{X}")
                    b_s = wk.tile([128, 64], F32, tag=f"b{X}")
                    n16 = wk.tile([128, 64], F16, tag=f"n{X}")
                    # t1 in f16 keeps the u-path math bitwise identical to the
                    # validated single-recurrence kernel (final max-err of this
                    # 200-step feedback loop is sensitive to rounding placement)
                    t1 = wk.tile([128, 4, HB], F16, tag=f"t1{X}")
                    u16 = wk.tile([128, 4, HB], F16, tag=f"u{X}")
                    e16 = wk.tile([128, 4, HB], F16, tag=f"e{X}")
                    h_t = hist[:, :, t, hb0:hb0 + HB]

                    # chain for step t (banks were filled by iteration t-1).
                    # critical loop: e_rz mms -> sig_r -> a -> b -> tanh ->
                    # e16 -> (next burst's e mms).  The u-path (sig_z1m ->
                    # t1 -> u on Pool) runs in parallel off-loop; splitting
                    # the sigmoid lets the r half feed the chain 54ns
                    # earlier (both halves read the same completed bank)
                    nc.scalar.activation(rz_s[:, 0:64], RZ[X][:, 0:64], SIG)
                    nc.scalar.activation(rz_s[:, 64:128], RZ[X][:, 64:128],
                                         SIG)
                    z1m4 = rz_s[:, 64:128].rearrange("p (q b) -> p q b", q=4)
                    # u path on Pool: t1 = (1-z)*h in f32, u = h - t1
                    nc.gpsimd.tensor_mul(t1, z1m4, h_t)
                    nc.gpsimd.tensor_sub(u16, h_t, t1)
                    # e path on DVE/Act; e16 in c-halves so the next
                    # burst's e_rz(c01) mms start early; the hist join
                    # rides the DVE queue after e16 (off the critical loop)
                    nc.vector.tensor_mul(a_s, rz_s[:, 0:64], NI[X][:, 0:64])
                    nc.vector.tensor_add(b_s, a_s, NI[X][:, 64:128])
                    nc.scalar.activation(n16, b_s, TANH)
                    n4 = n16.rearrange("p (q b) -> p q b", q=4)
                    nc.vector.tensor_mul(e16[:, 0:2, :], z1m4[:, 0:2, :],
                                         n4[:, 0:2, :])
                    nc.vector.tensor_mul(e16[:, 2:4, :], z1m4[:, 2:4, :],
                                         n4[:, 2:4, :])
                    nc.vector.tensor_add(hist[:, :, t + 1, hb0:hb0 + HB],
                                         u16, e16)

                    # burst for G(t+1): bias + u-operand mms first (u is
                    # ready early via the Pool path), then the e-operand
                    # mms rz-first (c01 then c23) so sig(t+1) fires after
                    # only the 32 e_rz mms
                    if t + 1 < steps:
                        u4 = [u16[:, c, :] for c in range(4)]
                        e4 = [e16[:, c, :] for c in range(4)]
                        bias_u_mms(X, ("r", "z", "hn", "in"), bt, wst, u4)
                        e_mms(X, ("r", "z"), e4, (), cs=(0, 1))
                        e_mms(X, ("r", "z"), e4, ("z",), cs=(2, 3))
                        e_mms(X, ("hn", "in"), e4, ("in",))

                    if next_proj[X] + PROJ_EVERY <= t:
                        proj_rows(X, next_proj[X], PROJ_EVERY)
                        next_proj[X] += PROJ_EVERY

                # stream finalized output chunks once BOTH halves are done
                while dma_done + DMA_CHUNK < min(next_proj):
                    lo = dma_done
                    nc.sync.dma_start(
                        out=out_d[:, :, lo:lo + DMA_CHUNK].rearrange(
                            "b v t -> v b t"),
                        in_=stage.rearrange("p (b t) -> p b t", b=BD)[
                            :, :, lo:lo + DMA_CHUNK])
                    dma_done += DMA_CHUNK

            for X in (0, 1):
                while next_proj[X] <= steps:
                    nrows = min(PROJ_EVERY, steps + 1 - next_proj[X])
                    proj_rows(X, next_proj[X], nrows)
                    next_proj[X] += nrows

        if dma_done < steps:
            nc.sync.dma_start(
                out=out_d[:, :, dma_done:].rearrange("b v t -> v b t"),
                in_=stage.rearrange("p (b t) -> p b t", b=BD)[:, :, dma_done:])
    nc.compile()
    return nc


_CACHE = {}


def _get_nc(steps: int):
    if steps not in _CACHE:
        _CACHE[steps] = _build(steps)
    return _CACHE[steps]


def _prep_inputs(feat, embed_table, w_ih, w_hh, b_ih, b_hh, proj_w, proj_b):
    f32 = np.float32
    f16 = np.float16
    w_ih = np.asarray(w_ih, f32)
    w_hh = np.asarray(w_hh, f32)
    b_ih = np.asarray(b_ih, f32)
    b_hh = np.asarray(b_hh, f32)
    # fused gate weights, gate-major order r, z, hn, in
    # z gate negated: sigmoid(z psum) then directly equals 1 - z
    Wc = np.concatenate([w_ih[:H] + w_hh[:H],
                         -(w_ih[H:2 * H] + w_hh[H:2 * H]),
                         w_hh[2 * H:],
                         w_ih[2 * H:]], 0)          # [4H, H]
    bc = np.concatenate([b_ih[:H] + b_hh[:H],
                         -(b_ih[H:2 * H] + b_hh[H:2 * H]),
                         b_hh[2 * H:],
                         b_ih[2 * H:]], 0)          # [4H]

    x0 = np.asarray(embed_table, f32)[0]
    gi0 = w_ih @ x0 + b_ih                          # [3H]
    bc0 = np.concatenate([gi0[:H] + b_hh[:H],
                          -(gi0[H:2 * H] + b_hh[H:2 * H]),
                          b_hh[2 * H:],
                          gi0[2 * H:]], 0)          # [4H]
    W0 = np.concatenate([w_hh[:H], -w_hh[H:2 * H]], 0)  # [2H, H] r,z step-0

    # stationary blocks: wst[kp, ((g*4+q)*4+c)*128 + m] = Wc[g*512+q*128+m,
    #                                                        c*128+kp]
    wst = np.empty((128, 64 * 128), f32)
    for g in range(4):
        for q in range(4):
            for c in range(4):
                blk = ((g * 4 + q) * 4 + c) * 128
                wst[:, blk:blk + 128] = Wc[g * 512 + q * 128:
                                           g * 512 + (q + 1) * 128,
                                           c * 128:(c + 1) * 128].T
    wst0 = np.empty((128, 32 * 128), f32)
    for g in range(2):
        for q in range(4):
            for c in range(4):
                blk = ((g * 4 + q) * 4 + c) * 128
                wst0[:, blk:blk + 128] = W0[g * 512 + q * 128:
                                            g * 512 + (q + 1) * 128,
                                            c * 128:(c + 1) * 128].T

    proj_w = np.asarray(proj_w, f32)                # [V, H]
    pjt = np.empty((128, 4 * VOCAB), f32)
    for c in range(4):
        pjt[:, c * VOCAB:(c + 1) * VOCAB] = proj_w[:, c * 128:(c + 1) * 128].T

    feat = np.asarray(feat, f32)
    common = {
        "wst": wst.astype(f16),
        "wst0": wst0.astype(f16),
        "bt": bc.reshape(1, 2048).astype(f16),
        "bt0": bc0.reshape(1, 2048).astype(f16),
        "ones": np.ones((1, BD), f16),
        "pjt": pjt.astype(f16),
        "pjb": np.asarray(proj_b, f32).reshape(VOCAB, 1),
    }
    maps = []
    for i in range(NCORES):
        fs = feat[i * BD:(i + 1) * BD]              # [BD, H]
        h0g = np.ascontiguousarray(
            fs.T.reshape(4, 128, BD).transpose(1, 0, 2).reshape(128, 128))
        maps.append(dict(common, h0=h0g.astype(f16)))
    return maps


def kernel(feat, embed_table, w_ih, w_hh, b_ih, b_hh, proj_w, proj_b,
           _trace=False):
    nc = _get_nc(STEPS)
    in_maps = _prep_inputs(feat, embed_table, w_ih, w_hh, b_ih, b_hh,
                           proj_w, proj_b)
    res = run_bass_kernel_spmd(nc, in_maps, list(range(NCORES)), trace=_trace)
    out = np.concatenate([res.results[i]["out"] for i in range(NCORES)], 0)
    if _trace:
        kernel.last_exec_time_ns = res.exec_time_ns
        kernel.last_results = res
    return out
